# revision 65
# baseline (speedup 1.0000x reference)
"""AdditiveScorer Trainium2 kernel — separable low-rank tanh expansion, v3.

logits[b,q,k] = W2 . tanh(keys[b,k] @ W1[:D] + queries[b,q] @ W1[D:] + b1) + b2
B=2, NQ=NK=1024, D=512, H=32.

Math (as v1, offline-fitted rank-(8x8) expansion):
    tanh(u+v) ~= sum_{j<8, l<8} A[j,l] * tanh((u-mu_j)/wq) * tanh((v-nu_l)/wk)
so  logits[q,k] = sum_{(j,h)} Phi[(j,h), q] * Ktil[(j,h), k]   (+ b2 on host)
with Phi = tanh((hq - mu_j)/wq),  Ktil = (A (x) diag(W2)) Psi,
     Psi = tanh((hk + b1 - nu_l)/wk).

Schedule (8 cores: b = c//4, key-slab = c%4, 256 keys):
  - queries ship as fp8e3m4 (e3m4 suits N(0,1) data; W1 stays fp16 and the
    PE takes mixed fp16-stationary x fp8-moving), halving the largest DMA.
  - input DMAs split over TWO issue paths so transfers run gapless on the
    DMA engines: SP/HWDGE (blob1, blob2, qt1, qt3) and Pool/SWDGE-inline
    (qt0, qt2). SP is surgically removed from the entry barrier so its
    issues start at t~150ns; Pool's start right after the barrier.
  - k-chain first (hk -> psi -> mixing -> ktil), query chunks pipeline
    behind it; drains spread DVE/ACT; smix ships precomputed from host.
  - PSUM pools sized so neither the ph_q nor the pout rotation stalls on
    drains (pout packs both kc halves into one 2KB bank).
  - warmup matmul train pins the PE p-state ramp from t~100ns.
(kv_writeback/trigger_dma SWDGE outputs would cut another ~1us off the
output tail but fail neuronxcc codegen in this build — "ISA wrong
length"; the USE_KVWB machinery is kept for a future toolchain.)
"""

import ml_dtypes
import numpy as np

import concourse.bass as bass
import concourse.tile as tile
from concourse import mybir
from concourse.bass_utils import run_bass_kernel_spmd

F32 = mybir.dt.float32
F16 = mybir.dt.float16
F8 = mybir.dt.float8e3

B, NQ, NK, D, H = 2, 1024, 1024, 512, 32
N_CORES = 8
KSLAB = NK // 4          # keys per core

# ---- offline-fitted separable expansion constants (mu/nu fp16-exact) ----
WQ = 1.0
WK = 0.8
MU = [-2.353515625, -1.6640625, -0.97509765625, -0.28564453125,
      0.403564453125, 1.0927734375, 1.7822265625, 2.470703125]
NU = [-1.888671875, -1.2587890625, -0.62841796875, 0.0016765594482421875,
      0.6318359375, 1.26171875, 1.8916015625, 2.521484375]
A_FIT = [
    [-2.2922555413, 2.7854862782, -1.8431440253, 0.1247330321, 2.1960775130, -4.8959156743, 6.6528257206, -4.2824348612],
    [2.2173366262, -2.6225874726, 1.4233224082, 0.9356560662, -4.4022028125, 7.7593600612, -7.1755681048, 3.3707667143],
    [-1.2804084212, 1.3189812492, 0.1224719852, -3.0964932672, 6.7179450858, -6.9212427042, 3.9662370603, -1.7020092772],
    [0.5124509553, 0.0460856534, -2.4173700365, 6.0478862590, -6.7227825466, 4.0054738413, -1.8071627054, 0.7393585903],
    [0.3961899499, -2.2540385816, 5.8460037826, -6.9598636224, 4.4486988003, -1.9671612294, 0.5584678999, -0.0437646090],
    [-2.2253882409, 6.0361718565, -7.6310309471, 5.3838337347, -2.6088505477, 0.5693082033, 0.6649752440, -0.8973553110],
    [5.1767203125, -8.1259470622, 6.5647064259, -3.7438531869, 1.1153604514, 1.0662968051, -2.6469182443, 2.6615612530],
    [-4.3513629055, 5.0839125923, -3.6428829637, 1.6759689775, 0.3998444280, -2.4318643113, 4.1362382133, -3.9812787610],
]

# ---- tunables -----------------------------------------------------------
QT_FP8 = True
QCH = [256, 256, 256, 256]       # query chunk sizes (sum = NQ; kv_writeback
                                 # needs 2*s to be a power of two)
QOFF = [sum(QCH[:i]) for i in range(len(QCH) + 1)]
QT_QUEUE = ["pool", "sp", "pool", "sp"]   # input DMA issue queue per chunk
DRAIN_ENG = ["dve", "dve", "act", "dve"]  # per-chunk drain engine
USE_KVWB = False                 # SWDGE outputs: walrus rejects the
                                 # trigger encoding ("ISA wrong length")
N_WARMUP = 194                   # PE p-state pin train (total)
N_WARMUP_PRE = 24                # hoisted ahead of the entry barrier
HOIST = True
STRIP_CONST_MEMSETS = True
SP_BARRIER_SURGERY = True

# blob1 columns (fp16): W1K unreplicated | KT0 | BVEC — thin, lands first;
# W1K is DVE-replicated off the critical path (the binder is blob2's sem)
OFF_W1K = 0
OFF_KT0 = 128
OFF_BVEC = 640
BLOB1 = OFF_BVEC + 8             # 648
# blob2: KT1 | W1Q (unrep) | SC | MASK
OFF_KT1 = 0
OFF_W1Q = 512
OFF_SC = 640
OFF_MASK = 656
BLOB2 = OFF_MASK + 128           # 784
W1Q_F8_COLS = 0


def _fix_kvwb_sync(nc, drain_names, trigger_names, prep_names, sem_names):
    """Tile wires the early kv_writeback preps as immediate readers of the
    osb bytes: each drain gets an inverted WAR wait on a writeback DMA sem
    (DMASW*, never updated — the descriptors carry our own sems), and the
    triggers end up mis-ordered/waiting on those sems. Repair: drop every
    dangling DMASW wait; give trigger_i explicit waits (drain_i done via its
    engine-sem tick, prep_i done via the Pool-sem tick); move the triggers
    and our completion-wait EventSemaphores to the end of Pool's stream so
    they neither run early nor head-of-line block Pool's other work."""
    f = nc.m.functions[0]
    insts = [i for blk in f.blocks for i in blk.instructions]
    by_name = {i.name: i for i in insts}
    updated_ids = set()
    sem_ids = {}
    for i in insts:
        si = i.sync_info
        if not si:
            continue
        for u in si.on_update:
            updated_ids.add(u.id)
            sem_ids.setdefault(str(u.ant_name), u.id)
    dangling = {w.id for i in insts if i.sync_info
                for w in i.sync_info.on_wait
                if w.id not in updated_ids and str(w.ant_name).startswith("DMASW")}

    # drop every dangling wait
    for i in insts:
        si = i.sync_info
        if not si:
            continue
        si.on_wait = [w for w in si.on_wait if w.id not in dangling]

    def tick_of(sem_id, upto):
        t = 0
        for i in insts:
            si = i.sync_info
            if si:
                for u in si.on_update:
                    if u.id == sem_id:
                        t += u.update_value
            if i is upto:
                break
        return t

    def eng_sem(inst):
        ups = [u for u in inst.sync_info.on_update
               if str(u.ant_name).startswith(inst.engine.name)]
        assert len(ups) == 1, (inst.name, ups)
        return ups[0]

    si_cls = type(next(i.sync_info for i in insts if i.sync_info is not None))
    move = []
    for dn, tn, pn in zip(drain_names, trigger_names, prep_names):
        drain, trig, prep = by_name[dn], by_name[tn], by_name[pn]
        du = eng_sem(drain)
        pu = eng_sem(prep)
        if trig.sync_info is None:
            trig.sync_info = si_cls(on_wait=[], on_update=[])
        # prep-before-trigger is already guaranteed by Pool program order;
        # only the drain-completion wait is needed (one wait also avoids a
        # NoOp carrier, which walrus rejects on the Pool engine).
        del pu
        trig.sync_info.on_wait = [
            mybir.SyncWait(sync_type="semaphore", id=du.id,
                           ant_name=du.ant_name, wait_mode="sem-ge-imm",
                           wait_value=tick_of(du.id, drain), wait_reg=None),
        ]
        move.append(trig)
    for sn in sem_names:
        sid = sem_ids.get(sn)
        waiter = next(i for i in insts
                      if i.engine == mybir.EngineType.Pool
                      and i.opcode == "EventSemaphore" and i.sync_info
                      and any(x.id == sid for x in i.sync_info.on_wait))
        move.append(waiter)
    for blk in f.blocks:
        blk.instructions = [i for i in blk.instructions if i not in move]
    for blk in reversed(f.blocks):
        at = None
        for idx, i in enumerate(blk.instructions):
            if i.engine == mybir.EngineType.Pool and i.opcode == "Drain":
                at = idx
                break
        if at is not None:
            blk.instructions[at:at] = move
            break


def _strip_final_barriers(nc):
    """The function epilogue carries TWO full all-engine barriers
    (TileContext exit + Bass finalize): ~490ns of gather/release
    choreography after the last DMA sem. Each engine's stream already ends
    with its own Drain (flush) behind per-engine DMA-completion waits, and
    NEFF completion is all-sequencers-halted — the cross-engine sync adds
    nothing. Remove the barrier EventSemaphores outside the entry block;
    the Drains' release==0 waits stay trivially true (the entry barrier
    returns the sem to zero) and the unconsumed gather increments are
    harmless."""
    f = nc.m.functions[0]
    for blk in f.blocks[1:]:
        blk.instructions = [
            i for i in blk.instructions
            if not (i.opcode == "EventSemaphore"
                    and str(i.name).startswith("barrier_"))
        ]


def _strip_redundant_self_waits(nc):
    """Tile chains same-engine instructions with waits on the engine's own
    completion sem. Engine FIFO order already guarantees them, but the sem
    only fires after the ~185ns pipelined write-ack + sem propagation, so a
    tight back-to-back pair stalls ~190ns. Drop any wait on the instruction's
    own engine sem whose value is implied by the increments of instructions
    earlier in the same engine stream (DMA completion sems are unaffected —
    DMAs update their own DMAHW/DMASW sems, which we never touch here)."""
    f = nc.m.functions[0]
    streams = {}
    for blk in f.blocks:
        for inst in blk.instructions:
            streams.setdefault(inst.engine, []).append(inst)
    for engine, insts in streams.items():
        prefix = "Pool" if engine == mybir.EngineType.Pool else engine.name
        # engine-sem ids and the running inc count along this stream
        counts = {}
        for inst in insts:
            si = inst.sync_info
            if not si:
                continue
            keep = []
            for w in si.on_wait:
                if (str(w.ant_name).startswith(prefix + "_")
                        and w.wait_mode == "sem-ge-imm"
                        and w.wait_value is not None
                        and counts.get(w.id, 0) >= w.wait_value):
                    continue
                keep.append(w)
            si.on_wait = keep
            for u in si.on_update:
                nm = str(u.ant_name)
                if nm.startswith(prefix + "_") and u.update_value:
                    counts[u.id] = counts.get(u.id, 0) + u.update_value


def _split_multi_waits(nc):
    """walrus rejects >1 sync wait per instruction; hoist extras onto
    single-wait NoOp carriers just before it in the same engine stream."""
    for f in nc.m.functions:
        for blk in f.blocks:
            out = []
            changed = False
            for inst in blk.instructions:
                si = inst.sync_info
                waits = list(si.on_wait) if si is not None else []
                if len(waits) > 1:
                    si_cls = type(si)
                    for j, w in enumerate(waits[:-1]):
                        nop = mybir.InstNoOp(name=f"{inst.name}-w{j}", ins=[], outs=[])
                        nop.engine = inst.engine
                        nop.sync_info = si_cls(on_wait=[w], on_update=[])
                        out.append(nop)
                    si.on_wait = [waits[-1]]
                    changed = True
                out.append(inst)
            if changed:
                blk.instructions = out


def _strip_const_memsets(nc):
    """Drop the framework const-pool memsets (serialize on Pool ahead of the
    entry barrier) when nothing reads those tensors."""
    for f in nc.m.functions:
        for blk in f.blocks:
            for inst in blk.instructions:
                if getattr(inst, "opcode", "") == "Memset":
                    continue
                for ap in list(inst.ins) + list(inst.outs):
                    mr = getattr(ap, "memref", None)
                    if mr is not None and str(mr).startswith("const-"):
                        return False
    for f in nc.m.functions:
        for blk in f.blocks:
            blk.instructions = [
                i for i in blk.instructions
                if not (getattr(i, "opcode", "") == "Memset" and i.outs
                        and str(getattr(i.outs[0], "memref", "")).startswith("const-"))
            ]
    return True


def _sp_barrier_surgery(nc):
    """Remove SP AND Pool from the ENTRY barrier: both streams are pure DMA
    issues into our own input tiles, which nothing in the preamble touches,
    so neither needs to participate. SP's Drain+EventSemaphore are deleted
    (its DMA issues then start at t~50ns); Pool's Drain is deleted and its
    two coordinator EventSemaphores (gather-wait, release-add) move to DVE,
    inserted right after DVE's Drain. Thresholds drop 4 -> 3 (ACT+PE+DVE)."""
    f = nc.m.functions[0]
    pre = f.blocks[0]
    out = []
    coord = []
    for inst in pre.instructions:
        if inst.opcode == "Drain" and inst.engine in (
                mybir.EngineType.SP, mybir.EngineType.Pool):
            continue
        if (inst.engine == mybir.EngineType.SP
                and inst.opcode == "EventSemaphore"
                and str(getattr(inst, "name", "")).startswith("barrier_SP")):
            continue
        si = inst.sync_info
        if (inst.engine == mybir.EngineType.Pool
                and inst.opcode == "EventSemaphore" and si is not None):
            for w in si.on_wait:
                if "gather" in str(w.ant_name) and w.wait_value == 4:
                    w.wait_value = 3
            for u in si.on_update:
                if "gather" in str(u.ant_name) and u.update_value == 4:
                    u.update_value = 3
                if "release" in str(u.ant_name) and u.update_value == 4:
                    u.update_value = 3
            inst.engine = mybir.EngineType.DVE
            coord.append(inst)
            continue
        out.append(inst)
    # place the coordinator pair right after DVE's Drain, before DVE's own
    # release-wait EventSemaphore
    at = None
    for idx, inst in enumerate(out):
        if inst.engine == mybir.EngineType.DVE and inst.opcode == "Drain":
            at = idx + 1
            break
    assert at is not None and len(coord) == 2
    out[at:at] = coord
    pre.instructions = out
    # SP's input DMA issues ahead of its RegisterMoves (static APs, no reg
    # dependence): HWDGE starts at ~50ns instead of ~325ns.
    first_sp = None
    for idx, inst in enumerate(pre.instructions):
        if inst.engine == mybir.EngineType.SP:
            first_sp = idx
            break
    main = f.blocks[1]
    dmas, rest = [], []
    for inst in main.instructions:
        if (inst.engine == mybir.EngineType.SP and inst.opcode == "DMACopy"
                and not (inst.sync_info and inst.sync_info.on_wait)):
            dmas.append(inst)
        else:
            rest.append(inst)
    main.instructions = rest
    if first_sp is None:
        first_sp = 0
    pre.instructions[first_sp:first_sp] = dmas


def _hoist_pre_barrier(nc, n_warm_pre):
    """Move the warmup-seed memset (DVE) and the first warmup
    Ldweights/Matmult pairs (PE) ahead of each engine's barrier Drain, so
    the PE p-state ramp clock starts at t~100ns."""
    f = nc.m.functions[0]
    blocks = f.blocks
    if len(blocks) < 2:
        return
    pre, main = blocks[0], blocks[1]

    def first_idx(blk, engine, opcode=None):
        for i, inst in enumerate(blk.instructions):
            if inst.engine == engine and (opcode is None or inst.opcode == opcode):
                return i
        return None

    def hoist(pred, count=None):
        taken, rest, n = [], [], 0
        for inst in main.instructions:
            if (count is None or n < count) and pred(inst):
                taken.append(inst)
                n += 1
            else:
                rest.append(inst)
        main.instructions = rest
        return taken

    seed = hoist(lambda i: i.engine == mybir.EngineType.DVE
                 and i.opcode == "Memset", count=1)
    warm = hoist(lambda i: i.engine == mybir.EngineType.PE
                 and i.opcode in ("Ldweights", "Matmult"),
                 count=2 * n_warm_pre)
    for group, engine in ((seed, mybir.EngineType.DVE),
                         (warm, mybir.EngineType.PE)):
        if not group:
            continue
        at = first_idx(pre, engine, "Drain")
        if at is None:
            at = len(pre.instructions)
        pre.instructions[at:at] = group


def _build_program():
    nc = bass.Bass()

    qdt = F8 if QT_FP8 else F16
    qex = W1Q_F8_COLS if QT_FP8 else W1Q_F8_COLS // 2
    blob1_d = nc.dram_tensor("blob1", [128, BLOB1], F16, kind="ExternalInput")
    blob2_d = nc.dram_tensor("blob2", [128, BLOB2], F16, kind="ExternalInput")
    qt_d = nc.dram_tensor("qt8", [128, 4 * NQ], qdt, kind="ExternalInput")
    o_d = nc.dram_tensor("o16", [1, 128, 1, 2 * NQ], F16, kind="ExternalOutput")

    nch = len(QCH)
    out_sems, osbr_h, osba_h = [], [], []
    if USE_KVWB:
        # two names per SBUF region: the drain writes osbr{i} while the
        # early kv_writeback descriptor prep references osba{i} — keeping
        # the prep free of data deps (the DMA reads the bytes only at
        # trigger time, after the drain; ordering restored in
        # _fix_kvwb_sync).
        off = nc.SBUF_PARTITION_SIZE_BYTES - 4 * 2 * max(QCH) * 2
        for ch in range(nch):
            s = QCH[ch]
            out_sems.append(nc.alloc_semaphore(f"out{ch}_dma"))
            osbr_h.append(nc.alloc_sbuf_tensor_at(
                f"osbr{ch}", [128, 2, s], F16, offset=off))
            osba_h.append(nc.alloc_sbuf_tensor_at(
                f"osba{ch}", [128, 1, 1, 2 * s], F16, offset=off))
            off += 4 * s

    with tile.TileContext(nc) as tc:
        with (
            tc.tile_pool(name="consts", bufs=1) as consts,
            tc.tile_pool(name="feats", bufs=1) as feats,
            tc.tile_pool(name="pk", bufs=1, space="PSUM") as pk,
            tc.tile_pool(name="pmix", bufs=1, space="PSUM") as pmix,
            tc.tile_pool(name="pq", bufs=3, space="PSUM") as pq,
            tc.tile_pool(name="pom", bufs=3, space="PSUM") as pom,
        ):
            # ---- input DMAs: SP-queue ones run from t~150 (SP exits the
            # barrier immediately); Pool-queue ones issue post-barrier.
            blob1 = consts.tile([128, BLOB1], F16, tag="blob1")
            nc.sync.dma_start(blob1[:], blob1_d[:])
            blob2 = consts.tile([128, BLOB2], F16, tag="blob2")
            nc.sync.dma_start(blob2[:], blob2_d[:])
            qtch = []
            for ch in range(nch):
                s, o = QCH[ch], QOFF[ch]
                t = consts.tile([128, 4, s], qdt, name=f"qt{ch}",
                                tag=f"qt{ch}")
                eng = nc.gpsimd if QT_QUEUE[ch] == "pool" else nc.sync
                eng.dma_start(t[:], qt_d[:, 4 * o:4 * (o + s)]
                              .rearrange("p (c s) -> p c s", c=4))
                qtch.append(t)

            def qdata(ch):
                return qtch[ch]

            bvec = blob1[:, OFF_BVEC:OFF_BVEC + 8].bitcast(F32)  # [128, 4]

            # ---- PE p-state warmup train (into pktil; mixing reads it
            # late enough that the WAW dep is already satisfied) ----
            wt = consts.tile([128, 16], F16, tag="wt")
            nc.vector.memset(wt[:], 0.0)
            pktil = pmix.tile([128, 2, 256], F32, tag="pktil")
            for _ in range(N_WARMUP):
                nc.tensor.matmul(pktil[0:16, 0, 0:16], wt[:], wt[:],
                                 start=True, stop=True)

            # W1K ships pre-replicated (it gates the k-chain — no DVE step);
            # W1Q rides qt0 unreplicated and is replicated on DVE (the BIR
            # verifier rejects stride-0 weight APs; DVE broadcast reads are
            # fine). Ready just before hq0 needs it.
            w1rep = feats.tile([128, 2, 4, 4, 32], F16, tag="w1rep")
            with tc.high_priority():
                for side, src_ap in ((0, blob1[:, OFF_W1K:OFF_W1K + 128]),
                                     (1, blob2[:, OFF_W1Q:OFF_W1Q + 128])):
                    nc.vector.tensor_copy(
                        w1rep[:, side],
                        src_ap.rearrange("p (c o h) -> p c o h", c=4, o=1)
                        .to_broadcast([128, 4, 4, 32]))

            def w1k(c):
                return w1rep[:, 0, c].rearrange("p g h -> p (g h)")

            def w1q(c):
                return w1rep[:, 1, c].rearrange("p g h -> p (g h)")

            def kt(kc, c):
                if kc == 0:
                    return blob1[:, OFF_KT0 + c * 128:OFF_KT0 + (c + 1) * 128]
                return blob2[:, OFF_KT1 + c * 128:OFF_KT1 + (c + 1) * 128]

            # mixing stationary smix = mask * scales, built on DVE from the
            # small SC/MASK tables in blob1 (ready well before mixing)
            maskv = blob2[:, OFF_MASK:OFF_MASK + 128] \
                .rearrange("p (j h) -> p j h", j=4)
            smix = feats.tile([128, 2, 2, 128], F16, tag="smix")
            for jb in range(2):
                for t in range(2):
                    sc = blob2[:, OFF_SC + (jb * 2 + t) * 4:
                               OFF_SC + (jb * 2 + t) * 4 + 4] \
                        .to_broadcast([128, 4, 32])
                    nc.vector.tensor_tensor(
                        smix[:, jb, t, :].rearrange("p (j h) -> p j h", j=4),
                        maskv, sc, mybir.AluOpType.mult)

            # ---- k-side: hk (per kc) -> psi (2 ACT, bias port) ->
            # mixing -> ktil ----
            ph_k = pk.tile([128, 256], F32, tag="ph_k")
            with tc.high_priority():
                for kc in range(2):
                    for c in range(4):
                        nc.tensor.matmul(ph_k[:, kc * 128:(kc + 1) * 128],
                                         w1k(c), kt(kc, c),
                                         start=(c == 0), stop=(c == 3))
            psi = [feats.tile([128, 256], F16, name=f"psi{t}", tag=f"psi{t}")
                   for t in range(2)]
            with tc.high_priority():
                for t in range(2):
                    nc.scalar.activation(
                        psi[t][:], ph_k[:],
                        mybir.ActivationFunctionType.Tanh,
                        bias=bvec[:, 2 + t:3 + t], scale=1.0 / WK,
                    )
            with tc.high_priority():
                for jb in range(2):
                    for t in range(2):
                        nc.tensor.matmul(pktil[:, jb, :], smix[:, jb, t, :],
                                         psi[t][:],
                                         start=(t == 0), stop=(t == 1))
            ktil = feats.tile([128, 2, 256], F16, tag="ktil")
            with tc.high_priority():
                for jb in range(2):
                    nc.vector.tensor_copy(ktil[:, jb], pktil[:, jb])

            # SWDGE output descriptors, prepared early (one per chunk)
            prep_names = []
            if USE_KVWB:
                ctx = consts.tile([128, 1], mybir.dt.int32, tag="ctx")
                nc.vector.memset(ctx[:], 0)
                for ch in range(nch):
                    s, o = QCH[ch], QOFF[ch]
                    p = nc.gpsimd.kv_writeback(
                        o_d[:, :, :, 2 * o:2 * (o + s)], osba_h[ch].ap(),
                        ctx[:], prepare_only=True, sem=out_sems[ch])
                    prep_names.append(p.ins.name)

            # ---- per query chunk: hq -> phi -> main matmul -> drain -> out
            drain_names, trigger_names = [], []
            for ch in range(nch):
                s, o = QCH[ch], QOFF[ch]
                ph_q = pq.tile([128, 256], F32, name=f"ph_q{ch}", tag="ph_q")
                with tc.tile_wait_until(0.0040 + 0.0009 * ch):
                    for c in range(4):
                        nc.tensor.matmul(ph_q[:, 0:s], w1q(c),
                                         qdata(ch)[:, c, :],
                                         start=(c == 0), stop=(c == 3))
                phi = [feats.tile([128, s], F16, name=f"phi{t}_{ch}",
                                  tag=f"phi{t}_{ch}") for t in range(2)]
                for t in range(2):
                    hint = 0.0050 + 0.0008 * ch + 0.0004 * t
                    with tc.tile_wait_until(hint):
                        nc.scalar.activation(
                            phi[t][:], ph_q[:, 0:s],
                            mybir.ActivationFunctionType.Tanh,
                            bias=bvec[:, t:t + 1], scale=1.0 / WQ,
                        )
                if USE_KVWB:
                    osb = osbr_h[ch].ap()
                else:
                    osb = feats.tile([128, 2, s], F16, name=f"osb{ch}",
                                     tag=f"osb{ch}")[:]
                # [2, 256] fp32 = two 1KB halves inside one 2KB bank: neither
                # kc group's matmul output straddles a bank boundary
                pout = pom.tile([128, 2, 256], F32, name=f"pout{ch}",
                                tag="pout")
                for kc in range(2):
                    for jb in range(2):
                        nc.tensor.matmul(
                            pout[:, kc, 0:s],
                            ktil[:, jb, kc * 128:(kc + 1) * 128],
                            phi[jb][:],
                            start=(jb == 0), stop=(jb == 1),
                        )
                if DRAIN_ENG[ch] == "act":
                    d = nc.scalar.copy(osb, pout[:, :, 0:s])
                else:
                    d = nc.vector.tensor_copy(osb, pout[:, :, 0:s])
                if USE_KVWB:
                    drain_names.append(d.ins.name)
                    t = nc.gpsimd.trigger_dma(count=1)
                    trigger_names.append(t.ins.name)
                    nc.gpsimd.wait_ge(out_sems[ch], 16)
                else:
                    # ch2's output issues via Pool/SWDGE-inline so the last
                    # chunk's HWDGE stage isn't queued behind three others
                    oeng = nc.gpsimd if ch == 2 else nc.sync
                    oeng.dma_start(
                        o_d[0, :, 0, 2 * o:2 * (o + s)]
                        .rearrange("p (t s) -> p t s", t=2), osb)

    if USE_KVWB:
        _fix_kvwb_sync(nc, drain_names, trigger_names, prep_names,
                       [f"out{ch}_dma" for ch in range(nch)])
    if STRIP_CONST_MEMSETS:
        _strip_const_memsets(nc)
    _strip_redundant_self_waits(nc)
    _strip_final_barriers(nc)
    if SP_BARRIER_SURGERY:
        _sp_barrier_surgery(nc)
    if HOIST:
        _hoist_pre_barrier(nc, N_WARMUP_PRE)
    _split_multi_waits(nc)
    return nc


_PROGRAM_CACHE = {}


def build_in_maps(keys, queries, W1, b1, W2, b2):
    keys = np.asarray(keys, dtype=np.float32)
    queries = np.asarray(queries, dtype=np.float32)
    W1 = np.asarray(W1, dtype=np.float32)
    b1 = np.asarray(b1, dtype=np.float32)
    W2 = np.asarray(W2, dtype=np.float32)

    def pmaj(x):  # [512, n] -> [128, 4*n] partition-major fp16
        return x.reshape(4, 128, -1).transpose(1, 0, 2).reshape(128, -1) \
            .astype(np.float16)

    w1k = pmaj(W1[:D])                       # [128, 128] unreplicated
    w1q = pmaj(W1[D:])                       # [128, 128] unreplicated

    mu, nu, A = np.array(MU), np.array(NU), np.array(A_FIT)
    m = np.arange(128)
    bvec = np.zeros((128, 4), dtype=np.float32)
    for t in range(2):
        bvec[:, t] = -mu[t * 4 + m // 32] / WQ
        bvec[:, 2 + t] = (b1[m % 32] - nu[t * 4 + m // 32]) / WK
    mask = np.tile(np.eye(32, dtype=np.float32), (4, 4))
    sc = np.zeros((128, 4, 4), dtype=np.float32)
    p = np.arange(128)
    for jb in range(2):
        for t in range(2):
            for jl in range(4):
                sc[:, jb * 2 + t, jl] = A[jb * 4 + jl, t * 4 + p // 32] \
                    * W2[p % 32, 0]
    mask16 = mask.astype(np.float16)
    sc16 = sc.reshape(128, 16).astype(np.float16)

    qdt = ml_dtypes.float8_e3m4 if QT_FP8 else np.float16
    qtv = []     # [128, qex + 4*NQ]: W1Q (bitcast) | chunk-major query data
    for b in range(B):
        qT = queries[b].T.reshape(4, 128, NQ).transpose(1, 0, 2)  # [128,4,NQ]
        parts = [qT[:, :, QOFF[ch]:QOFF[ch + 1]].reshape(128, -1).astype(qdt)
                 for ch in range(len(QCH))]
        qtv.append(np.ascontiguousarray(np.concatenate(parts, axis=1)))

    in_maps = []
    for c in range(N_CORES):
        b, ks = divmod(c, 4)
        ksl = keys[b, ks * KSLAB:(ks + 1) * KSLAB].T   # [512, 256]
        ktm = pmaj(ksl).reshape(128, 4, 256)           # [128, 4, 256]
        kt0 = np.ascontiguousarray(ktm[:, :, 0:128]).reshape(128, -1)
        kt1 = np.ascontiguousarray(ktm[:, :, 128:256]).reshape(128, -1)
        blob1 = np.concatenate([w1k, kt0, bvec.view(np.float16)], axis=1)
        blob2 = np.concatenate([kt1, w1q, sc16, mask16], axis=1)
        assert blob1.shape[1] == BLOB1, blob1.shape
        assert blob2.shape[1] == BLOB2, blob2.shape
        in_maps.append({
            "blob1": np.ascontiguousarray(blob1),
            "blob2": np.ascontiguousarray(blob2),
            "qt8": qtv[b],
        })
    return in_maps


def kernel(keys, queries, W1, b1, W2, b2):
    if "nc" not in _PROGRAM_CACHE:
        _PROGRAM_CACHE["nc"] = _build_program()
    nc = _PROGRAM_CACHE["nc"]

    in_maps = build_in_maps(keys, queries, W1, b1, W2, b2)
    res = run_bass_kernel_spmd(nc, in_maps, list(range(N_CORES)))

    b2v = float(np.asarray(b2, dtype=np.float32)[0])
    out = np.empty((B, NQ, NK), dtype=np.float32)
    for c in range(N_CORES):
        b, ks = divmod(c, 4)
        o = res.results[c]["o16"].reshape(128, 2 * NQ).astype(np.float32)
        dst = out[b, :, ks * KSLAB:(ks + 1) * KSLAB]
        for ch in range(len(QCH)):
            s, of = QCH[ch], QOFF[ch]
            blk = o[:, 2 * of:2 * (of + s)].reshape(128, 2, s)
            dst[of:of + s] = blk.transpose(2, 1, 0).reshape(s, KSLAB) + b2v
    return out
